# revision 1
# baseline (speedup 1.0000x reference)
"""BipartiteSAGEConv on 8 Trainium2 NeuronCores.

out = normalize(mean_{dst}(x[src]) @ W_l + b_l + x @ W_r)

Strategy:
- Host: sort edges by destination node, shard destination-node ranges across
  the 8 cores (each core owns 12500 contiguous nodes and all edges pointing
  into them -> no cross-core reduction needed). Per 128-node tile, edges are
  grouped by src bank (4 banks of 25024 rows, since dma_gather indices are
  int16) and packed into KB chunks of 128 per bank (padded; padding edges
  carry dstrel=-1 so the one-hot kills them). The per-edge weight
  w = 1/max(deg(dst),1) is folded into the one-hot so the PSUM accumulation
  yields the mean directly.
- Device (SPMD, identical program on all 8 cores):
  * dma_gather (Ant SWDGE gather, int16 idx) of x[src] rows per (tile, bank)
  * DVE builds the weighted one-hot: (iota == dstrel) * w  (one fused op)
  * PE accumulates meanT[f, n] += msg[e, f].T @ onehot[e, n] over chunks
  * PE: out[n, fo] = meanT.T @ W_l + xT.T @ W_r + ones x b_l (one PSUM group)
  * ACT Square+accum -> row sum of squares; sqrt; clamp; DVE reciprocal;
    scale rows; DMA out.
"""

import numpy as np

N_NODES = 100000
D = 128
N_CORES = 8
NODES_PER_CORE = N_NODES // N_CORES  # 12500
P = 128
TILES_PER_CORE = (NODES_PER_CORE + P - 1) // P  # 98
NODE_PAD = TILES_PER_CORE * P  # 12544
X_PAD_ROWS = 100096  # 782 * 128; >= 7*12500 + 12544
BANK = X_PAD_ROWS // 4  # 25024 rows per gather bank (< 32768 int16 limit)
NBANKS = 4

_program_cache = {}

# test harness hooks
TRACE = False
LAST = {}

NQUEUES = 4  # SWDGE queues; gathers round-robin across them
SCRATCH = 16384  # SWDGE descriptor-ring carveout bytes (ring = SCRATCH // 16)
GATHER_BF16 = True  # gather from a bf16 copy of x (halves gather bytes);
SINGLE_PACKET = True  # dma_gather packet mode (HW ucode knob, A/B on HW)
# the one-hot is then exact 0/1 bf16 and the 1/deg scaling happens in f32
# on the (summed @ W_l) product instead of being folded into the one-hot.


def _build_program(KB: int, bench_repeat: int = 1, ablate: str = ""):
    """Build + compile the SPMD Bass program; KB = edge chunks per (tile, bank).

    bench_repeat > 1 wraps the main loop in a For_i that recomputes the same
    output bench_repeat times (for device-time measurement only).
    ablate: comma-set of {gather, onehot, chunkmm} to skip (bench only).
    """
    ablate_set = set(ablate.split(",")) if ablate else set()
    import contextlib

    import concourse.bass as bass
    import concourse.tile as tile
    from concourse import bacc, mybir
    from concourse.masks import make_identity

    f32 = mybir.dt.float32
    bf16 = mybir.dt.bfloat16
    i16 = mybir.dt.int16
    gdt = bf16 if GATHER_BF16 else f32
    KT = NBANKS * KB  # chunk slots per tile
    NIDX = KB * P  # indices per gather
    IW = NIDX // 16  # idx columns per (tile, bank)

    nc = bacc.Bacc(
        "TRN2",
        target_bir_lowering=False,
        debug=False,
        num_devices=N_CORES,
        num_swdge_queues=NQUEUES,
        dynamic_dma_scratch_size=SCRATCH,
    )

    if GATHER_BF16:
        xgat = nc.dram_tensor("xbf", [X_PAD_ROWS, D], bf16, kind="ExternalInput")
        rcol = nc.dram_tensor("rcol", [P, TILES_PER_CORE], f32, kind="ExternalInput")
    else:
        xgat = nc.dram_tensor("xpad", [X_PAD_ROWS, D], f32, kind="ExternalInput")
    xchunk = nc.dram_tensor("xchunk", [NODE_PAD, D], f32, kind="ExternalInput")
    gidx = nc.dram_tensor(
        "gidx", [P, TILES_PER_CORE, NBANKS, IW], i16, kind="ExternalInput"
    )
    dstrel = nc.dram_tensor("dstrel", [P, TILES_PER_CORE, KT], f32, kind="ExternalInput")
    wgt = nc.dram_tensor("wgt", [P, TILES_PER_CORE, KT], f32, kind="ExternalInput")
    wl = nc.dram_tensor("wl", [D, D], f32, kind="ExternalInput")
    wr = nc.dram_tensor("wr", [D, D], f32, kind="ExternalInput")
    bl = nc.dram_tensor("bl", [1, D], f32, kind="ExternalInput")
    out = nc.dram_tensor("out", [NODE_PAD, D], f32, kind="ExternalOutput")

    with tile.TileContext(nc) as tc:
        with (
            tc.tile_pool(name="const", bufs=1) as const_pool,
            tc.tile_pool(name="meta", bufs=1) as meta_pool,
            tc.tile_pool(name="msg", bufs=3 * NBANKS) as msg_pool,
            tc.tile_pool(name="oh", bufs=6) as oh_pool,
            tc.tile_pool(name="xt", bufs=3) as xt_pool,
            tc.tile_pool(name="ep", bufs=3) as ep_pool,
            tc.tile_pool(name="ps_mean", bufs=2, space="PSUM") as ps_mean_pool,
            tc.tile_pool(name="ps_xt", bufs=2, space="PSUM") as ps_xt_pool,
            tc.tile_pool(name="ps_out", bufs=2, space="PSUM") as ps_out_pool,
        ):
            # ---- constants / weights / metadata (loaded once) ----
            iota_i = const_pool.tile([P, P], mybir.dt.int32)
            nc.gpsimd.iota(iota_i[:], pattern=[[1, P]], base=0, channel_multiplier=0)
            iota_f = const_pool.tile([P, P], f32)
            nc.vector.tensor_copy(iota_f[:], iota_i[:])
            if GATHER_BF16:
                iota_g = const_pool.tile([P, P], bf16)
                nc.vector.tensor_copy(iota_g[:], iota_i[:])
                rcol_sb = const_pool.tile([P, TILES_PER_CORE], f32)
                nc.sync.dma_start(rcol_sb[:], rcol[:])
            else:
                iota_g = iota_f

            identity = const_pool.tile([P, P], f32)
            make_identity(nc, identity[:])

            wl_sb = const_pool.tile([D, D], f32)
            nc.sync.dma_start(wl_sb[:], wl[:])
            wr_sb = const_pool.tile([D, D], f32)
            nc.sync.dma_start(wr_sb[:], wr[:])
            bl_sb = const_pool.tile([1, D], f32)
            nc.sync.dma_start(bl_sb[:], bl[:])
            ones1 = const_pool.tile([1, D], f32)
            nc.vector.memset(ones1[:], 1.0)

            idx_all = meta_pool.tile([P, TILES_PER_CORE, NBANKS, IW], i16)
            nc.sync.dma_start(idx_all[:], gidx[:])
            dst_all = meta_pool.tile([P, TILES_PER_CORE, KT], f32)
            nc.sync.dma_start(dst_all[:], dstrel[:])
            w_all = meta_pool.tile([P, TILES_PER_CORE, KT], f32)
            nc.sync.dma_start(w_all[:], wgt[:])

            # ---- main loop over node tiles ----
            rep_ctx = (
                tc.For_i(0, bench_repeat, 1)
                if bench_repeat > 1
                else contextlib.nullcontext()
            )
            with rep_ctx:
              for t in range(TILES_PER_CORE):
                # gather x[src] per bank: msg position (p, j) <- edge j*128+p
                msgs = []
                if "gather" not in ablate_set:
                    nb = 2 if "2banks" in ablate_set else NBANKS
                    elem = D // 2 if "half" in ablate_set else D
                    estep = D if "half" in ablate_set else None
                    for b in range(nb):
                        xpad_v = xgat[b * BANK : (b + 1) * BANK, :elem]
                        msg = msg_pool.tile([P, KB, elem], gdt, tag="msg")
                        if "splitgather" in ablate_set:
                            k1 = 2  # chunks in first gather
                            nc.gpsimd.dma_gather(
                                out_ap=msg[:, :k1, :],
                                in_ap=xpad_v,
                                idxs_ap=idx_all[:, t, b, : k1 * 8],
                                num_idxs=k1 * P,
                                num_idxs_reg=k1 * P,
                                elem_size=elem,
                                elem_step=estep,
                                queue_num=b % NQUEUES,
                            )
                            nc.gpsimd.dma_gather(
                                out_ap=msg[:, k1:, :],
                                in_ap=xpad_v,
                                idxs_ap=idx_all[:, t, b, k1 * 8 :],
                                num_idxs=(KB - k1) * P,
                                num_idxs_reg=(KB - k1) * P,
                                elem_size=elem,
                                elem_step=estep,
                                queue_num=b % NQUEUES,
                            )
                        else:
                            nc.gpsimd.dma_gather(
                                out_ap=msg[:],
                                in_ap=xpad_v,
                                idxs_ap=idx_all[:, t, b, :],
                                num_idxs=NIDX,
                                num_idxs_reg=NIDX,
                                elem_size=elem,
                                elem_step=estep,
                                single_packet=SINGLE_PACKET,
                                queue_num=b % NQUEUES,
                            )
                        msgs.append(msg)

                # root path: x tile, transposed via PE
                x_sb = xt_pool.tile([P, D], f32, tag="x_in")
                nc.sync.dma_start(x_sb[:], xchunk[t * P : (t + 1) * P, :])
                ps_xt = ps_xt_pool.tile([P, P], f32)
                nc.tensor.transpose(out=ps_xt[:], in_=x_sb[:], identity=identity[:])
                xT_sb = xt_pool.tile([P, D], f32, tag="x_t")
                nc.scalar.copy(xT_sb[:], ps_xt[:])

                # aggregation: sumT/meanT[f, n] accumulated over chunk slots
                ps_mean = ps_mean_pool.tile([P, P], f32)
                for s in range(KT):
                    b, j = divmod(s, KB)
                    if "onehot" not in ablate_set:
                        oh = oh_pool.tile([P, P], gdt)
                        if GATHER_BF16:
                            nc.vector.tensor_scalar(
                                oh[:],
                                iota_g[:],
                                dst_all[:, t, s : s + 1],
                                None,
                                mybir.AluOpType.is_equal,
                            )
                        else:
                            nc.vector.tensor_scalar(
                                oh[:],
                                iota_g[:],
                                dst_all[:, t, s : s + 1],
                                w_all[:, t, s : s + 1],
                                mybir.AluOpType.is_equal,
                                mybir.AluOpType.mult,
                            )
                        rhs_ap = oh[:]
                    else:
                        rhs_ap = iota_g[:]
                    if "chunkmm" not in ablate_set:
                        lhs_ap = (
                            msgs[b][:, j, :]
                            if "gather" not in ablate_set
                            else iota_g[:]
                        )
                        nc.tensor.matmul(
                            out=ps_mean[:],
                            lhsT=lhs_ap,
                            rhs=rhs_ap,
                            start=(s == 0),
                            stop=(s == KT - 1),
                        )
                if "chunkmm" in ablate_set:
                    nc.tensor.matmul(
                        out=ps_mean[:],
                        lhsT=iota_g[:],
                        rhs=iota_g[:],
                        start=True,
                        stop=True,
                    )
                meanT_sb = ep_pool.tile([P, P], f32, tag="meanT")
                nc.scalar.copy(meanT_sb[:], ps_mean[:])

                if GATHER_BF16:
                    # ps_a = sumT.T @ W_l; scale rows by 1/deg (exact f32)
                    ps_a = ps_out_pool.tile([P, P], f32, tag="ps_a")
                    nc.tensor.matmul(
                        out=ps_a[:], lhsT=meanT_sb[:], rhs=wl_sb[:],
                        start=True, stop=True,
                    )
                    out_l = ep_pool.tile([P, P], f32, tag="out_l")
                    nc.vector.tensor_scalar(
                        out_l[:],
                        ps_a[:],
                        rcol_sb[:, t : t + 1],
                        None,
                        mybir.AluOpType.mult,
                    )
                    # ps_b = xT.T @ W_r + ones x b_l; final = out_l + ps_b
                    ps_o = ps_out_pool.tile([P, P], f32, tag="ps_b")
                    nc.tensor.matmul(
                        out=ps_o[:], lhsT=xT_sb[:], rhs=wr_sb[:],
                        start=True, stop=False,
                    )
                    nc.tensor.matmul(
                        out=ps_o[:], lhsT=ones1[:], rhs=bl_sb[:],
                        start=False, stop=True,
                    )
                    final = ep_pool.tile([P, P], f32, tag="final")
                    nc.vector.tensor_tensor(
                        out=final[:], in0=out_l[:], in1=ps_o[:],
                        op=mybir.AluOpType.add,
                    )
                    norm_src = final[:]
                else:
                    # linear: out[n,fo] = meanT.T @ W_l + xT.T @ W_r + ones x b_l
                    ps_o = ps_out_pool.tile([P, P], f32, tag="ps_b")
                    nc.tensor.matmul(
                        out=ps_o[:], lhsT=meanT_sb[:], rhs=wl_sb[:],
                        start=True, stop=False,
                    )
                    nc.tensor.matmul(
                        out=ps_o[:], lhsT=xT_sb[:], rhs=wr_sb[:],
                        start=False, stop=False,
                    )
                    nc.tensor.matmul(
                        out=ps_o[:], lhsT=ones1[:], rhs=bl_sb[:],
                        start=False, stop=True,
                    )
                    norm_src = ps_o[:]

                # row-wise L2 normalize: out / max(||out||, 1e-12)
                sq_scr = ep_pool.tile([P, P], f32, tag="sq")
                ss = ep_pool.tile([P, 1], f32, tag="ss")
                nc.scalar.activation(
                    sq_scr[:],
                    norm_src,
                    mybir.ActivationFunctionType.Square,
                    accum_out=ss[:],
                )
                nrm = ep_pool.tile([P, 1], f32, tag="nrm")
                nc.scalar.sqrt(nrm[:], ss[:])
                nrmc = ep_pool.tile([P, 1], f32, tag="nrmc")
                nc.vector.tensor_scalar_max(nrmc[:], nrm[:], 1e-12)
                rn = ep_pool.tile([P, 1], f32, tag="rn")
                nc.vector.reciprocal(rn[:], nrmc[:])

                out_sb = ep_pool.tile([P, P], f32, tag="out")
                nc.vector.tensor_scalar(
                    out_sb[:],
                    norm_src,
                    rn[:, :1],
                    None,
                    mybir.AluOpType.mult,
                )
                nc.sync.dma_start(out[t * P : (t + 1) * P, :], out_sb[:])

    nc.compile()
    return nc


def _prepare(x, edge_index):
    """Host-side sharding: sort by dst, group per (tile, bank), pack chunks."""
    src = np.ascontiguousarray(edge_index[0]).astype(np.int64)
    dst = np.ascontiguousarray(edge_index[1]).astype(np.int64)

    cnt = np.bincount(dst, minlength=N_NODES)
    w_node = (1.0 / np.maximum(cnt, 1)).astype(np.float32)

    order = np.argsort(dst, kind="stable")
    src_s = src[order]
    dst_s = dst[order]

    # per-core edge ranges and per (core,tile,bank) grouping
    per_core = []
    KB = 1
    for c in range(N_CORES):
        base = c * NODES_PER_CORE
        lo = np.searchsorted(dst_s, base)
        hi = np.searchsorted(dst_s, base + NODES_PER_CORE)
        s_c = src_s[lo:hi]
        d_c = dst_s[lo:hi] - base
        t_c = d_c // P
        b_c = s_c // BANK
        key = (t_c * NBANKS + b_c).astype(np.int64)
        ordc = np.argsort(key, kind="stable")
        s_c, d_c, key = s_c[ordc], d_c[ordc], key[ordc]
        counts = np.bincount(key, minlength=TILES_PER_CORE * NBANKS)
        KB = max(KB, int(np.ceil(counts.max() / P)))
        per_core.append((s_c, d_c, counts))

    KT = NBANKS * KB
    NIDX = KB * P
    IW = NIDX // 16

    # per-node 1/max(deg,1) as [core][lane, tile] columns
    wg = np.ones(X_PAD_ROWS, np.float32)
    wg[:N_NODES] = w_node
    rcol = np.zeros((N_CORES, P, TILES_PER_CORE), np.float32)
    for c in range(N_CORES):
        idx = (
            c * NODES_PER_CORE
            + (np.arange(TILES_PER_CORE) * P)[None, :]
            + np.arange(P)[:, None]
        )
        rcol[c] = wg[idx]

    gidx = np.zeros((N_CORES, P, TILES_PER_CORE, NBANKS, IW), np.int16)
    dstrel = np.full((N_CORES, P, TILES_PER_CORE, KT), -1.0, np.float32)
    wgt = np.zeros((N_CORES, P, TILES_PER_CORE, KT), np.float32)

    prow = np.arange(P) % 16
    scol = np.arange(IW) * 16
    for c in range(N_CORES):
        s_c, d_c, counts = per_core[c]
        starts = np.concatenate([[0], np.cumsum(counts)])
        for t in range(TILES_PER_CORE):
            for b in range(NBANKS):
                g = t * NBANKS + b
                n = counts[g]
                if n == 0:
                    continue
                lo = starts[g]
                sv = s_c[lo : lo + n] - b * BANK
                dv = (d_c[lo : lo + n] - t * P).astype(np.float32)
                wv = w_node[d_c[lo : lo + n] + c * NODES_PER_CORE]
                i_pad = np.zeros(NIDX, np.int16)
                i_pad[:n] = sv.astype(np.int16)
                d_pad = np.full(NIDX, -1.0, np.float32)
                d_pad[:n] = dv
                w_pad = np.zeros(NIDX, np.float32)
                w_pad[:n] = wv
                # idx position i lives at [i % 16, i // 16], replicated %16
                gidx[c, :, t, b, :] = i_pad[scol[None, :] + prow[:, None]]
                # chunk slot s=b*KB+j, lane p <- edge j*128+p
                dstrel[c, :, t, b * KB : (b + 1) * KB] = d_pad.reshape(KB, P).T
                wgt[c, :, t, b * KB : (b + 1) * KB] = w_pad.reshape(KB, P).T

    return gidx, dstrel, wgt, rcol, KB


def kernel(x, edge_index, W_l, b_l, W_r):
    from concourse.bass_utils import run_bass_kernel_spmd

    x = np.ascontiguousarray(np.asarray(x, dtype=np.float32))
    W_l = np.ascontiguousarray(np.asarray(W_l, dtype=np.float32))
    W_r = np.ascontiguousarray(np.asarray(W_r, dtype=np.float32))
    b_l = np.ascontiguousarray(np.asarray(b_l, dtype=np.float32)).reshape(1, D)

    gidx, dstrel, wgt, rcol, KB = _prepare(x, np.asarray(edge_index))

    xpad = np.zeros((X_PAD_ROWS, D), np.float32)
    xpad[:N_NODES] = x
    if GATHER_BF16:
        import ml_dtypes

        xbf = xpad.astype(ml_dtypes.bfloat16)

    if KB not in _program_cache:
        _program_cache[KB] = _build_program(KB)
    nc = _program_cache[KB]

    in_maps = []
    for c in range(N_CORES):
        base = c * NODES_PER_CORE
        m = {
            "xchunk": xpad[base : base + NODE_PAD],
            "gidx": gidx[c],
            "dstrel": dstrel[c],
            "wgt": wgt[c],
            "wl": W_l,
            "wr": W_r,
            "bl": b_l,
        }
        if GATHER_BF16:
            m["xbf"] = xbf
            m["rcol"] = rcol[c]
        else:
            m["xpad"] = xpad
        in_maps.append(m)

    LAST["nc"] = nc
    LAST["in_maps"] = in_maps
    r = run_bass_kernel_spmd(nc, in_maps, list(range(N_CORES)), trace=TRACE)
    LAST["exec_time_ns"] = r.exec_time_ns
    res = r.results
    out = np.concatenate(
        [res[c]["out"][:NODES_PER_CORE] for c in range(N_CORES)], axis=0
    )
    return out



# revision 13
# speedup vs baseline: 99.5452x; 99.5452x over previous
"""BipartiteSAGEConv on 8 Trainium2 NeuronCores.

out = normalize(mean_{dst}(x[src]) @ W_l + b_l + x @ W_r)

Sharding: edges sorted by destination node; each core owns 12500
contiguous destination nodes and every edge pointing into them, so no
cross-core reduction is needed. Per 128-node destination tile, edges are
grouped by source bank (4 banks of 25024 rows — dma_gather indices are
int16) and packed into C chunks of 128 edges per (tile, bank).

Per-core device program (SPMD, identical on all 8 cores):
  * dma_gather (SWDGE, int16 idx) pulls x[src] rows (bf16) per
    (tile, bank) — 4 queues, one call per bank. This paces the kernel:
    the drain rate is ~2.3ns per 256B descriptor.
  * DVE builds all KT=4C one-hot chunks of a tile in ONE broadcast
    tensor_tensor is_equal op: oh[p, s, c] = (iota[c] == dstrel[p, s]).
    Padding lanes carry dstrel=-1000 so their one-hot row is zero.
  * PE accumulates sumT[f, n] += msg_s[e, f].T @ oh_s[e, n] over the KT
    chunk slots in PSUM (bf16 operands, f32 accumulate).
  * Epilogue per tile (ACT-heavy, DVE-light):
      meanT copy (ACT) -> ps_a = sumT.T @ W_l (PE, f32)
      out_l = ps_a * rcol rows (ACT scale, exact f32 1/deg)
      ps_o = xT.T @ W_r + ones x b_l (PE, bf16; xT is a host-transposed
        bf16 resident slice - no on-device transposes)
      final = out_l + ps_o (DVE), row L2 norm via ACT Square+accum,
      sqrt (ACT), max+reciprocal (DVE), row scale (ACT), DMA out.
"""

import numpy as np

N_NODES = 100000
D = 128
P = 128
N_CORES = 8
NODES_PER_CORE = N_NODES // N_CORES  # 12500
TILES_PER_CORE = (NODES_PER_CORE + P - 1) // P  # 98
NODE_PAD = TILES_PER_CORE * P  # 12544
X_PAD_ROWS = 100096  # 782 * 128; >= 7*12500 + 12544
BANK = X_PAD_ROWS // 4  # 25024 rows per gather bank (int16 safe)
NBANKS = 4

NQUEUES = 4
SCRATCH = 49152  # SWDGE descriptor-ring carveout bytes per partition
PAD_REL = -1000.0  # one-hot miss value for padding lanes

_program_cache = {}

# test harness hooks
TRACE = False
LAST = {}


def _build_program(C: int):
    """Build + compile the SPMD Bass program; C = chunks per (tile, bank)."""
    import concourse.bass as bass
    import concourse.tile as tile
    from concourse import bacc, mybir

    f32 = mybir.dt.float32
    bf16 = mybir.dt.bfloat16
    i16 = mybir.dt.int16
    KT = NBANKS * C  # chunk slots per tile
    NIDX = C * P  # indices per gather call
    IW = NIDX // 16  # idx columns per (tile, bank)

    nc = bacc.Bacc(
        "TRN2",
        target_bir_lowering=False,
        debug=False,
        num_devices=N_CORES,
        num_swdge_queues=NQUEUES,
        dynamic_dma_scratch_size=SCRATCH,
    )

    xbf = nc.dram_tensor("xbf", [X_PAD_ROWS, D], bf16, kind="ExternalInput")
    xTc = nc.dram_tensor("xTc", [D, NODE_PAD], bf16, kind="ExternalInput")
    gidx = nc.dram_tensor(
        "gidx", [P, TILES_PER_CORE, NBANKS, IW], i16, kind="ExternalInput"
    )
    dstrel = nc.dram_tensor(
        "dstrel", [P, TILES_PER_CORE * KT, 1], f32, kind="ExternalInput"
    )
    rcol = nc.dram_tensor("rcol", [P, TILES_PER_CORE], f32, kind="ExternalInput")
    wl = nc.dram_tensor("wl", [D, D], f32, kind="ExternalInput")
    wrb = nc.dram_tensor("wrb", [D, D], bf16, kind="ExternalInput")
    blb = nc.dram_tensor("blb", [1, D], bf16, kind="ExternalInput")
    out = nc.dram_tensor("out", [NODE_PAD, D], f32, kind="ExternalOutput")

    with tile.TileContext(nc) as tc:
        with (
            tc.tile_pool(name="const", bufs=1) as const_pool,
            tc.tile_pool(name="meta", bufs=1) as meta_pool,
            tc.tile_pool(name="msg", bufs=3 * NBANKS) as msg_pool,
            tc.tile_pool(name="oh", bufs=3) as oh_pool,
            tc.tile_pool(name="ep", bufs=4) as ep_pool,
            tc.tile_pool(name="ps_mean", bufs=2, space="PSUM") as ps_mean_pool,
            tc.tile_pool(name="ps_a", bufs=2, space="PSUM") as ps_a_pool,
            tc.tile_pool(name="ps_o", bufs=2, space="PSUM") as ps_o_pool,
        ):
            # ---- constants / weights / metadata (loaded once) ----
            iota_i = const_pool.tile([P, 1, P], mybir.dt.int32)
            nc.gpsimd.iota(iota_i[:], pattern=[[1, P]], base=0, channel_multiplier=0)
            iota3 = const_pool.tile([P, 1, P], bf16)
            nc.vector.tensor_copy(iota3[:], iota_i[:])

            wl_sb = const_pool.tile([D, D], f32)
            nc.sync.dma_start(wl_sb[:], wl[:])
            wr_sb = const_pool.tile([D, D], bf16)
            nc.sync.dma_start(wr_sb[:], wrb[:])
            bl_sb = const_pool.tile([1, D], bf16)
            nc.sync.dma_start(bl_sb[:], blb[:])
            ones1 = const_pool.tile([1, D], bf16)
            nc.vector.memset(ones1[:], 1.0)
            rcol_sb = const_pool.tile([P, TILES_PER_CORE], f32)
            nc.sync.dma_start(rcol_sb[:], rcol[:])

            xT_sb = meta_pool.tile([D, NODE_PAD], bf16)
            nc.sync.dma_start(xT_sb[:], xTc[:])
            idx_all = meta_pool.tile([P, TILES_PER_CORE, NBANKS, IW], i16)
            nc.sync.dma_start(idx_all[:], gidx[:])
            dst_all = meta_pool.tile([P, TILES_PER_CORE * KT, 1], f32)
            nc.sync.dma_start(dst_all[:], dstrel[:])

            # ---- main loop over destination-node tiles ----
            for t in range(TILES_PER_CORE):
                # gather x[src] per bank: msg position (p, j) <- edge j*128+p
                msgs = []
                for b in range(NBANKS):
                    msg = msg_pool.tile([P, C, D], bf16, tag="msg")
                    nc.gpsimd.dma_gather(
                        out_ap=msg[:],
                        in_ap=xbf[b * BANK : (b + 1) * BANK, :],
                        idxs_ap=idx_all[:, t, b, :],
                        num_idxs=NIDX,
                        num_idxs_reg=NIDX,
                        elem_size=D,
                        single_packet=True,
                        queue_num=b % NQUEUES,
                    )
                    msgs.append(msg)

                # all KT one-hots of this tile in one broadcast DVE op
                oh = oh_pool.tile([P, KT, P], bf16, tag="oh")
                nc.vector.tensor_tensor(
                    out=oh[:],
                    in0=iota3[:].to_broadcast((P, KT, P)),
                    in1=dst_all[:, t * KT : (t + 1) * KT, :].to_broadcast(
                        (P, KT, P)
                    ),
                    op=mybir.AluOpType.is_equal,
                )

                # aggregation: sumT[f, n] accumulated over chunk slots
                ps_mean = ps_mean_pool.tile([P, P], f32)
                for s in range(KT):
                    b, j = divmod(s, C)
                    nc.tensor.matmul(
                        out=ps_mean[:],
                        lhsT=msgs[b][:, j, :],
                        rhs=oh[:, s, :],
                        start=(s == 0),
                        stop=(s == KT - 1),
                    )
                meanT_sb = ep_pool.tile([P, P], f32, tag="meanT")
                nc.scalar.copy(meanT_sb[:], ps_mean[:])

                # ps_a = sumT.T @ W_l ; scale rows by 1/deg (exact f32)
                ps_a = ps_a_pool.tile([P, P], f32)
                nc.tensor.matmul(
                    out=ps_a[:], lhsT=meanT_sb[:], rhs=wl_sb[:],
                    start=True, stop=True,
                )
                out_l = ep_pool.tile([P, P], f32, tag="out_l")
                nc.scalar.activation(
                    out_l[:],
                    ps_a[:],
                    mybir.ActivationFunctionType.Copy,
                    scale=rcol_sb[:, t : t + 1],
                )

                # ps_o = xT.T @ W_r + ones x b_l (bf16 path)
                ps_o = ps_o_pool.tile([P, P], f32)
                nc.tensor.matmul(
                    out=ps_o[:],
                    lhsT=xT_sb[:, t * P : (t + 1) * P],
                    rhs=wr_sb[:],
                    start=True,
                    stop=False,
                )
                nc.tensor.matmul(
                    out=ps_o[:], lhsT=ones1[:], rhs=bl_sb[:],
                    start=False, stop=True,
                )
                final = ep_pool.tile([P, P], f32, tag="final")
                nc.vector.tensor_tensor(
                    out=final[:], in0=out_l[:], in1=ps_o[:],
                    op=mybir.AluOpType.add,
                )

                # row-wise L2 normalize: out / max(||out||, 1e-12)
                sq_scr = ep_pool.tile([P, P], f32, tag="sq")
                ss = ep_pool.tile([P, 1], f32, tag="ss")
                nc.scalar.activation(
                    sq_scr[:],
                    final[:],
                    mybir.ActivationFunctionType.Square,
                    accum_out=ss[:],
                )
                nrm = ep_pool.tile([P, 1], f32, tag="nrm")
                nc.scalar.sqrt(nrm[:], ss[:])
                nrmc = ep_pool.tile([P, 1], f32, tag="nrmc")
                nc.vector.tensor_scalar_max(nrmc[:], nrm[:], 1e-12)
                rn = ep_pool.tile([P, 1], f32, tag="rn")
                nc.vector.reciprocal(rn[:], nrmc[:])

                out_sb = ep_pool.tile([P, P], f32, tag="out")
                nc.scalar.activation(
                    out_sb[:],
                    final[:],
                    mybir.ActivationFunctionType.Copy,
                    scale=rn[:, :1],
                )
                nc.sync.dma_start(out[t * P : (t + 1) * P, :], out_sb[:])

    nc.compile()
    return nc


def _prepare(x, edge_index):
    """Host-side sharding: sort by dst, group per (tile, bank), pack chunks."""
    src = np.ascontiguousarray(edge_index[0]).astype(np.int64)
    dst = np.ascontiguousarray(edge_index[1]).astype(np.int64)

    cnt = np.bincount(dst, minlength=N_NODES)
    w_node = (1.0 / np.maximum(cnt, 1)).astype(np.float32)

    order = np.argsort(dst, kind="stable")
    src_s = src[order]
    dst_s = dst[order]

    # per-core edge ranges and per (core,tile,bank) grouping
    per_core = []
    C = 1
    for c in range(N_CORES):
        base = c * NODES_PER_CORE
        lo = np.searchsorted(dst_s, base)
        hi = np.searchsorted(dst_s, base + NODES_PER_CORE)
        s_c = src_s[lo:hi]
        d_c = dst_s[lo:hi] - base
        t_c = d_c // P
        b_c = s_c // BANK
        key = (t_c * NBANKS + b_c).astype(np.int64)
        ordc = np.argsort(key, kind="stable")
        s_c, d_c, key = s_c[ordc], d_c[ordc], key[ordc]
        counts = np.bincount(key, minlength=TILES_PER_CORE * NBANKS)
        C = max(C, int(np.ceil(counts.max() / P)))
        per_core.append((s_c, d_c, counts))

    KT = NBANKS * C
    NIDX = C * P
    IW = NIDX // 16

    # per-node 1/max(deg,1) as [core][lane, tile] columns
    wg = np.ones(X_PAD_ROWS, np.float32)
    wg[:N_NODES] = w_node
    rcol = np.zeros((N_CORES, P, TILES_PER_CORE), np.float32)
    for c in range(N_CORES):
        idx = (
            c * NODES_PER_CORE
            + (np.arange(TILES_PER_CORE) * P)[None, :]
            + np.arange(P)[:, None]
        )
        rcol[c] = wg[idx]

    gidx = np.zeros((N_CORES, P, TILES_PER_CORE, NBANKS, IW), np.int16)
    dstrel = np.full((N_CORES, P, TILES_PER_CORE, KT), PAD_REL, np.float32)
    gcnt = np.full((N_CORES, 1, TILES_PER_CORE * NBANKS), 16, np.int32)

    prow = np.arange(P) % 16
    scol = np.arange(IW) * 16
    for c in range(N_CORES):
        s_c, d_c, counts = per_core[c]
        starts = np.concatenate([[0], np.cumsum(counts)])
        for t in range(TILES_PER_CORE):
            for b in range(NBANKS):
                g = t * NBANKS + b
                n = counts[g]
                if n == 0:
                    continue
                lo = starts[g]
                gcnt[c, 0, g] = max(16, ((int(n) + 15) // 16) * 16)
                sv = s_c[lo : lo + n] - b * BANK
                dv = (d_c[lo : lo + n] - t * P).astype(np.float32)
                i_pad = np.zeros(NIDX, np.int16)
                i_pad[:n] = sv.astype(np.int16)
                d_pad = np.full(NIDX, PAD_REL, np.float32)
                d_pad[:n] = dv
                # idx position i lives at [i % 16, i // 16], replicated %16
                gidx[c, :, t, b, :] = i_pad[scol[None, :] + prow[:, None]]
                # chunk slot s=b*C+j, lane p <- edge j*128+p
                dstrel[c, :, t, b * C : (b + 1) * C] = d_pad.reshape(C, P).T

    return gidx, dstrel.reshape(N_CORES, P, TILES_PER_CORE * KT, 1), rcol, gcnt, C


def kernel(x, edge_index, W_l, b_l, W_r):
    import ml_dtypes
    from concourse.bass_utils import run_bass_kernel_spmd

    x = np.ascontiguousarray(np.asarray(x, dtype=np.float32))
    W_l = np.ascontiguousarray(np.asarray(W_l, dtype=np.float32))
    W_r = np.asarray(W_r, dtype=np.float32)
    b_l = np.asarray(b_l, dtype=np.float32).reshape(1, D)

    gidx, dstrel, rcol, gcnt, C = _prepare(x, np.asarray(edge_index))

    xpad = np.zeros((X_PAD_ROWS, D), np.float32)
    xpad[:N_NODES] = x
    xbf = xpad.astype(ml_dtypes.bfloat16)
    wrb = W_r.astype(ml_dtypes.bfloat16)
    blb = b_l.astype(ml_dtypes.bfloat16)

    if C not in _program_cache:
        _program_cache[C] = _build_program(C)
    nc = _program_cache[C]

    in_maps = []
    for c in range(N_CORES):
        base = c * NODES_PER_CORE
        xTc = np.ascontiguousarray(xbf[base : base + NODE_PAD].T)
        in_maps.append(
            {
                "xbf": xbf,
                "xTc": xTc,
                "gidx": gidx[c],
                "dstrel": dstrel[c],
                "rcol": rcol[c],
                "wl": W_l,
                "wrb": wrb,
                "blb": blb,
            }
        )

    LAST["nc"] = nc
    LAST["in_maps"] = in_maps
    r = run_bass_kernel_spmd(nc, in_maps, list(range(N_CORES)), trace=TRACE)
    LAST["exec_time_ns"] = r.exec_time_ns
    res = r.results
    out = np.concatenate(
        [res[c]["out"][:NODES_PER_CORE] for c in range(N_CORES)], axis=0
    )
    return out
